# revision 21
# baseline (speedup 1.0000x reference)
"""Trainium2 Bass kernel for nn_DAttention:
out[b,c,d,h,w] = x[b,c,d,h,w] * mean_{c,h,w}(x[b,:,d,:,:]).

Sharding: weighted data parallel over the 256 (b,d) slices (2 MiB each).
The 8 NeuronCores share ~2.8 TB/s of chip HBM bandwidth, but the HBM/NOC
arbitration is statically unfair: under full 8-core contention, jax
devices 0/1 (physical nc4/nc5) sustain only ~250-280 GB/s while devices
3/4/5/7 get the full per-core ~430 GB/s DMA line rate (measured from
all-core NTFF profiles). Uniform B-sharding therefore leaves the starved
cores running ~150 us past the rest — and max-core time is what counts.

Fix: assign each core a slice count proportional to its measured
contended bandwidth. The host packs each core's slices contiguously
(x transposed to [B,D,C,H,W] -> 256 x 2 MiB slices), so every DMA is a
fully sequential HBM stream. One SPMD NEFF runs on all cores; the
per-core count is derived on-device from partition_id() (a register
TensorLoad of an arbitrary input tensor dies on the axon/PJRT path, but
the partition-id load is plumbed specially and works), and the unequal
trip counts are realized with predicated DMAs (cond= skips the transfer
but still bumps the semaphore). Real slices sit at the HEAD of the
S_MAX iteration range: head-skips stall the real load stream behind the
skipped iterations' garbage compute (they hold xin pool buffers for
~2.9 us of ACT work each — measured 15-57 us of head gaps), while
tail-skips overlap that garbage compute with the trailing store phase,
costing only the light-loaded cores (which have slack) a few us.
The multiply is done in place (DVE reads and writes xt), freeing the
separate output pool's SBUF for a deeper load-ahead window.

Per-slice engine schedule (one big op per engine per iteration):
  ACT: two activation-Copies (halves) into a dead PSUM scratch with
       accum_out -> per-partition column sums (no SBUF write traffic)
  PE : two accumulated matmuls against a constant 128x128 matrix of
       1/524288 -> cross-partition sum + broadcast of the mean
  ACT: tiny copy of the mean PSUM->SBUF
  DVE: single tensor_scalar multiply (2x fp32 mode)
  ACT: store DMA issue (predicated)
"""
import numpy as np

import concourse.bacc as bacc
import concourse.tile as tile
import concourse.mybir as mybir
from concourse.bass_utils import run_bass_kernel_spmd

B, C, D, H, W = 8, 32, 32, 128, 128
P = 128                 # SBUF partitions
F = 4096                # free elements per partition; P*F = one (b,d) slice
N_RED = C * H * W       # 524288 = 2**19 elements reduced per (b, d)
RECIP = 1.0 / N_RED     # exact in fp32
NSLICES = B * D         # 256

# Slices per jax device, proportional to measured contended HBM bandwidth
# (device order 0..7 = physical nc 4,5,6,7,2,3,0,1). Sum must be 256.
COUNTS = [28, 30, 31, 35, 32, 35, 30, 35]
assert sum(COUNTS) == NSLICES
S_MAX = max(COUNTS)     # compiled loop bound
MIN_COUNT = min(COUNTS)  # iterations below this run unconditionally

_NC = None


def _build_nc(xin_bufs=10):
    nc = bacc.Bacc("TRN2", target_bir_lowering=False, debug=False)
    x5 = nc.dram_tensor("x", [S_MAX, P, F], mybir.dt.float32, kind="ExternalInput")
    o5 = nc.dram_tensor("out", [S_MAX, P, F], mybir.dt.float32, kind="ExternalOutput")
    half = F // 2

    def real_count(eng):
        # n = COUNTS[pid]; iteration s runs iff s < n
        pid = eng.partition_id()
        t = None
        for i in range(8):
            term = (pid == i) * COUNTS[i]
            t = term if t is None else t + term
        return eng.snap(t, min_val=0, max_val=S_MAX)

    with tile.TileContext(nc) as tc:
        with (
            tc.tile_pool(name="xin", bufs=xin_bufs) as xpool,
            tc.tile_pool(name="small", bufs=6) as spool,
            tc.tile_pool(name="psum", bufs=2, space="PSUM") as ppool,
            tc.tile_pool(name="psc", bufs=1, space="PSUM") as scpool,
            tc.tile_pool(name="const", bufs=1) as cpool,
        ):
            # recip is a NEFF-baked Const, DMA'd on the store ring (idle at
            # the head) so the first slice load on the sync ring isn't
            # delayed; gpsimd is never used.
            recip_dram = nc.inline_tensor(
                np.full((P, P), RECIP, dtype=np.float32), name="recipc"
            )
            recip = cpool.tile([P, P], mybir.dt.float32)
            nc.scalar.dma_start(recip[:], recip_dram[:, :])
            # Per-core real-iteration count: computed lazily right before its
            # first use (iteration MIN_COUNT) so the ~1.6us register
            # TensorLoad doesn't sit ahead of the first DMAs.
            n_sync = n_scl = None
            for s in range(S_MAX):
                # real slices occupy s in [0, n); skip the tail. The first
                # MIN_COUNT iterations are real on every core, so their DMAs
                # skip the predication register math entirely.
                xt = xpool.tile([P, F], mybir.dt.float32, tag="xt")
                if s < MIN_COUNT:
                    nc.sync.dma_start(xt[:], x5[s])
                else:
                    if n_sync is None:
                        n_sync = real_count(nc.sync)
                        n_scl = real_count(nc.scalar)
                    nc.sync.dma_start(xt[:], x5[s], cond=(n_sync > s))
                csa = spool.tile([P, 1], mybir.dt.float32, tag="csa")
                csb = spool.tile([P, 1], mybir.dt.float32, tag="csb")
                scratch = scpool.tile([P, half], mybir.dt.float32, tag="sc")
                nc.scalar.activation(
                    scratch[:], xt[:, :half],
                    mybir.ActivationFunctionType.Copy, accum_out=csa[:],
                )
                nc.scalar.activation(
                    scratch[:], xt[:, half:],
                    mybir.ActivationFunctionType.Copy, accum_out=csb[:],
                )
                dv = ppool.tile([P, 1], mybir.dt.float32, tag="dv")
                nc.tensor.matmul(dv[:], recip[:], csa[:], start=True, stop=False)
                nc.tensor.matmul(dv[:], recip[:], csb[:], start=False, stop=True)
                # DVE reads the broadcast mean straight from PSUM
                nc.vector.tensor_scalar_mul(xt[:], xt[:], dv[:])
                if s < MIN_COUNT:
                    nc.scalar.dma_start(o5[s], xt[:])
                else:
                    nc.scalar.dma_start(o5[s], xt[:], cond=(n_scl > s))
    nc.compile()
    return nc


def _get_nc():
    global _NC
    if _NC is None:
        _NC = _build_nc()
    return _NC


def run(x: np.ndarray, trace: bool = False, tmpdir: str | None = None):
    """Run on 8 NeuronCores; returns (out, BassKernelResults)."""
    x = np.asarray(x)
    assert x.shape == (B, C, D, H, W), x.shape
    x = x.astype(np.float32, copy=False)
    nc = _get_nc()
    # [B,C,D,H,W] -> [B,D,C,H,W] -> 256 slices of [C*H*W]; slice (b,d) is
    # contiguous so each core's shard is a pure sequential HBM stream.
    xt = np.ascontiguousarray(x.transpose(0, 2, 1, 3, 4)).reshape(NSLICES, P, F)
    offs = np.concatenate([[0], np.cumsum(COUNTS)])
    in_maps = []
    for i in range(8):
        n = COUNTS[i]
        xp = np.empty((S_MAX, P, F), dtype=np.float32)
        xp[:n] = xt[offs[i]:offs[i + 1]]
        in_maps.append({"x": xp})
    res = run_bass_kernel_spmd(
        nc, in_maps, core_ids=list(range(8)), trace=trace, tmpdir=tmpdir
    )
    ot = np.empty((NSLICES, P, F), dtype=np.float32)
    for i in range(8):
        n = COUNTS[i]
        ot[offs[i]:offs[i + 1]] = res.results[i]["out"][:n]
    out = ot.reshape(B, D, C, H, W).transpose(0, 2, 1, 3, 4)
    return np.ascontiguousarray(out), res


def kernel(x: np.ndarray) -> np.ndarray:
    out, _ = run(x)
    return out
